# revision 4
# baseline (speedup 1.0000x reference)
"""Trainium2 kernel for the bilinear form y[b,k] = sum_ij x[b,i] x[b,j] W[i,j,k] + b[k].

Shapes: x (512, 784) f32, W (614656=784*784, 10) f32, b (10,) f32 -> y (512, 10) f32.

Strategy (8 NeuronCores):
  - Shard the j axis of W.reshape(784, 784, 10) across cores: 98 j's per core.
    Each core reads W/8 + full x (~2.5 MB in fp16); compute is the long pole.
  - Stage 1 (TensorE): U[b, (k,j)] = sum_i x[b,i] * W[i, j_shard, k], x^T tiles
    stationary, W shard moving, accumulating over 7 uniform 112-row i-tiles
    into 8 PSUM banks (4 batch tiles x 2 column halves), i-tile-major so the
    PE chases DMA arrivals; the last 2 i-tiles run group-major so PSUM groups
    stop staggered and stage 2 overlaps the remaining matmuls.
  - Stage 2 (DVE + Pool, alternating per group): fused multiply+reduce via
    scalar_tensor_tensor accum_out: y[b, k] = sum_j U[b, (k,j)] * x[b, j],
    one 98-wide call per (group, k).
  - Host: y = sum_c y_part_c + b  (20 KB per core; no collectives needed).

Perf notes:
  - Matmul operands are fp16 (fp32 is 4x slower on the PE; DMA halves).
    fp32 PSUM accumulation keeps the overall error ~1e-3.
  - All DRAM inputs are partition-major with >=1 KB contiguous runs per
    partition row so the HW DGE streams near peak; W is chunked per i-tile
    (1.96 KB rows) on the sync ring, x^T/xs on the scalar ring, ordered by
    first consumption.
  - Dummy warmup matmuls (into PSUM bank 0, discarded by the first real
    start=True matmul) keep the PE busy while the first DMAs land, so the
    HAM clock gate reaches 2.4 GHz early.
  - y accumulates in one SBUF tile and leaves in a single contiguous DMA.
"""

import numpy as np

D = 784
B = 512
C = 10
NCORES = 8
JS = D // NCORES  # 98 j's per core
JK = JS * C  # 980 free columns per core, laid out as (k, j)
HALF = JK // 2  # 490 = 5 k's x 98 j's -> one PSUM bank
P = 128
B_TILES = B // P  # 4
IT = 7  # i-tiles
IP = D // IT  # 112 rows per i-tile (uniform, no padding)
N_WARMUP_MM = 4  # dummy matmuls (N=HALF) that warm the PE clock gate
PREFIX_ITS = 5  # i-tiles issued i-tile-major; the rest run group-major
XT_CHUNKS = [(0, 1), (1, 3), (3, 7)]  # xT DMA chunks (i-tile ranges)

MM_DTYPE = "float16"  # dtype of the matmul operands (and their DMA)

_nc_cache = {}


def _build_nc():
    import concourse.bacc as bacc
    import concourse.mybir as mybir
    import concourse.tile as tile

    mm_dt = getattr(mybir.dt, MM_DTYPE)
    f32 = mybir.dt.float32

    nc = bacc.Bacc("TRN2", target_bir_lowering=False)

    # Partition-major DRAM layouts (see _make_in_maps).
    xT = nc.dram_tensor("xT", [IP, IT, B], mm_dt, kind="ExternalInput")
    w = nc.dram_tensor("w", [IP, IT, 2, HALF], mm_dt, kind="ExternalInput")
    xs = nc.dram_tensor("xs", [P, B_TILES, JS], f32, kind="ExternalInput")
    y = nc.dram_tensor("y", [P, B_TILES * C], f32, kind="ExternalOutput")

    with tile.TileContext(nc) as tc:
        with (
            tc.tile_pool(name="wpool", bufs=8) as wpool,
            tc.tile_pool(name="xpool", bufs=3) as xpool,
            tc.tile_pool(name="xspool", bufs=1) as xspool,
            tc.tile_pool(name="ypool", bufs=1) as ypool,
            tc.tile_pool(name="scratch", bufs=6) as spool,
            tc.tile_pool(name="psum", bufs=8, space="PSUM") as psum_pool,
        ):
            # x^T it0 first on the scalar ring; w it0 split by half across
            # both rings so the very first matmul's operands land earliest.
            xT_sb = {}
            xt0 = xpool.tile([IP, 1, B], mm_dt, name="xt_c0", tag="xt0")
            nc.scalar.dma_start(xt0[:], xT[:, 0:1, :])
            xT_sb[0] = xt0[:, 0, :]

            w_sb = {}  # (it, h) -> [IP, HALF] view
            w0 = wpool.tile([IP, 1, 2, HALF], mm_dt, name="w_it0", tag="w")
            nc.sync.dma_start(w0[:, :, 0, :], w[:, 0:1, 0, :])
            nc.scalar.dma_start(w0[:, :, 1, :], w[:, 0:1, 1, :])
            w_sb[(0, 0)] = w0[:, 0, 0, :]
            w_sb[(0, 1)] = w0[:, 0, 1, :]

            # Remaining x^T chunks on the scalar ring.
            for c0, c1 in XT_CHUNKS[1:]:
                xt = xpool.tile([IP, c1 - c0, B], mm_dt, name=f"xt_c{c0}", tag="xt")
                nc.scalar.dma_start(xt[:], xT[:, c0:c1, :])
                for it in range(c0, c1):
                    xT_sb[it] = xt[:, it - c0, :]

            # Remaining w i-tiles on the sync ring, in consumption order.
            for it in range(1, IT):
                wt = wpool.tile([IP, 1, 2, HALF], mm_dt, name=f"w_it{it}", tag="w")
                nc.sync.dma_start(wt[:], w[:, it : it + 1, :, :])
                w_sb[(it, 0)] = wt[:, 0, 0, :]
                w_sb[(it, 1)] = wt[:, 0, 1, :]

            xs_sb = xspool.tile([P, B_TILES, JS], f32)
            nc.scalar.dma_start(xs_sb[:], xs[:])

            # PSUM: 8 accumulation groups (bt, h), one bank each. Warmups
            # write into group 0's bank; the first real start=True matmul
            # clears has_written so the garbage is discarded.
            pts = {}
            for bt in range(B_TILES):
                for h in range(2):
                    pts[(bt, h)] = psum_pool.tile(
                        [P, HALF], f32, name=f"pt_b{bt}h{h}", tag="pt", bufs=8
                    )

            # PE warmup: dummy matmuls with no DMA dependency keep the PE busy
            # while the first DMAs land, so the HAM clock gate warms early.
            dmy_s = spool.tile([IP, P], mm_dt, name="dmy_s", tag="dmy_s", bufs=1)
            dmy_m = spool.tile([IP, HALF], mm_dt, name="dmy_m", tag="dmy_m", bufs=1)
            nc.gpsimd.memset(dmy_s[:], 0.0)
            nc.gpsimd.memset(dmy_m[:], 0.0)
            for _ in range(N_WARMUP_MM):
                nc.tensor.matmul(
                    pts[(0, 0)][:], dmy_s[:], dmy_m[:], start=True, stop=True
                )

            y_t = ypool.tile([P, B_TILES * C], f32)

            def mm(it, bt, h, start, stop):
                nc.tensor.matmul(
                    pts[(bt, h)][:],
                    xT_sb[it][:, bt * P : (bt + 1) * P],
                    w_sb[(it, h)][:],
                    start=start,
                    stop=stop,
                )

            # Prefix i-tiles i-tile-major so the PE chases DMA arrivals...
            for it in range(PREFIX_ITS):
                for bt in range(B_TILES):
                    for h in range(2):
                        mm(it, bt, h, start=(it == 0), stop=False)

            # ...then group-major so the stop matmuls stagger and stage 2
            # (DVE + Pool) overlaps the remaining matmuls.
            groups = [(bt, h) for bt in range(B_TILES) for h in range(2)]
            for gi, (bt, h) in enumerate(groups):
                for it in range(PREFIX_ITS, IT):
                    mm(it, bt, h, start=False, stop=(it == IT - 1))
                pt = pts[(bt, h)]
                if gi % 2 == 0:
                    # DVE path: fused multiply+reduce straight from PSUM.
                    for kh in range(C // 2):
                        scr = spool.tile(
                            [P, JS], mybir.dt.bfloat16, name=f"scr{gi}_{kh}",
                            tag="scr_v", bufs=2,
                        )
                        k = h * (C // 2) + kh
                        nc.vector.scalar_tensor_tensor(
                            out=scr[:],
                            in0=pt[:, kh * JS : (kh + 1) * JS],
                            scalar=1.0,
                            in1=xs_sb[:, bt, :],
                            op0=mybir.AluOpType.mult,
                            op1=mybir.AluOpType.mult,
                            accum_out=y_t[:, bt * C + k : bt * C + k + 1],
                        )
                else:
                    # Act+Pool path: Pool can't read PSUM and has no
                    # free-axis reduce, so Act copies PSUM->SBUF, Pool does
                    # the xs multiply, and Act's accum_out does the j-sum.
                    ucopy = spool.tile(
                        [P, HALF], f32, name=f"ucopy{gi}", tag="ucopy", bufs=2
                    )
                    nc.scalar.activation(
                        ucopy[:], pt[:], mybir.ActivationFunctionType.Copy
                    )
                    vt = spool.tile([P, HALF], f32, name=f"vt{gi}", tag="vt", bufs=2)
                    u3 = ucopy[:].rearrange("p (kh j) -> p kh j", kh=C // 2)
                    v3 = vt[:].rearrange("p (kh j) -> p kh j", kh=C // 2)
                    xs3 = xs_sb[:, bt, None, :].broadcast_to([P, C // 2, JS])
                    nc.gpsimd.tensor_tensor(v3, u3, xs3, mybir.AluOpType.mult)
                    for kh in range(C // 2):
                        scr = spool.tile(
                            [P, JS], mybir.dt.bfloat16, name=f"scr{gi}_{kh}",
                            tag="scr_g", bufs=2,
                        )
                        k = h * (C // 2) + kh
                        nc.scalar.activation(
                            scr[:],
                            vt[:, kh * JS : (kh + 1) * JS],
                            mybir.ActivationFunctionType.Copy,
                            accum_out=y_t[:, bt * C + k : bt * C + k + 1],
                        )

            nc.sync.dma_start(y[:], y_t[:])

    nc.compile()
    return nc


def _get_nc():
    if "nc" not in _nc_cache:
        _nc_cache["nc"] = _build_nc()
    return _nc_cache["nc"]


def _make_in_maps(x, W):
    import concourse.mybir as mybir

    mm_np = mybir.dt.np(getattr(mybir.dt, MM_DTYPE))
    x = np.asarray(x, dtype=np.float32)
    Wr = np.asarray(W, dtype=np.float32).reshape(D, D, C)
    # xT_dram[p, it, b] = x[b, it*IP + p]
    xT = np.ascontiguousarray(
        x.T.astype(mm_np).reshape(IT, IP, B).transpose(1, 0, 2)
    )
    in_maps = []
    for c in range(NCORES):
        js, je = c * JS, (c + 1) * JS
        # wsh[i, k*JS + j] = Wr[i, js+j, k]; then [p, it, h, col] partition-major
        wsh = Wr[:, js:je, :].transpose(0, 2, 1).reshape(D, JK).astype(mm_np)
        wshard = np.ascontiguousarray(
            wsh.reshape(IT, IP, 2, HALF).transpose(1, 0, 2, 3)
        )
        # xs_dram[p, bt, j] = x[bt*P + p, js + j]
        xsl = np.ascontiguousarray(
            x[:, js:je].reshape(B_TILES, P, JS).transpose(1, 0, 2)
        )
        in_maps.append({"xT": xT, "w": wshard, "xs": xsl})
    return in_maps


def run_spmd(x, W, **spmd_kwargs):
    """Compile/run the SPMD kernel; returns (partials, BassKernelResults)."""
    from concourse.bass_utils import run_bass_kernel_spmd

    nc = _get_nc()
    in_maps = _make_in_maps(x, W)
    res = run_bass_kernel_spmd(nc, in_maps, core_ids=list(range(NCORES)), **spmd_kwargs)
    # y_dram[p, bt*C + k] -> y[bt*P + p, k]
    partials = [
        r["y"].reshape(P, B_TILES, C).transpose(1, 0, 2).reshape(B, C)
        for r in res.results
    ]
    return partials, res


def kernel(x, W, b):
    partials, _ = run_spmd(x, W)
    y = np.sum(np.stack(partials, 0), axis=0, dtype=np.float64) + np.asarray(
        b, dtype=np.float64
    )
    return y.astype(np.float32)
